# revision 5
# baseline (speedup 1.0000x reference)
"""Self-contained Trainium2 Bass kernel for nn_DisGNN (CGConv GNN), 8-core SPMD.

v2: marshal-lean variant. Each core computes h0/tables for its OWN nodes only;
the conv1 src table is assembled by an on-device AllGather of per-core table
rows, and conv2's table is computed from a chunked h1T AllGather that overlaps
conv1. Gather index slabs ship de-replicated ([16, n]) and expand on device.
"""
import sys, os
for p in ('/opt/trn_rl_repo', '/root/.axon_site/_ro/trn_rl_repo'):
    if os.path.isdir(p) and p not in sys.path:
        sys.path.insert(0, p)
import contextlib
import numpy as np

# ======================= host preprocessing =======================

N, E, C, D, NCLS, G = 50000, 800000, 128, 32, 10, 64
NCORE = 8
NPC = 6272              # nodes per core (49*128)
WPC = 49                # windows per core
NPAD = NCORE * NPC      # 50176
S0 = 32640              # src split (255*128): table A nodes [0,S0), B [S0,NPAD)
NB_TAB = NPAD // 128    # 392 table blocks
# window chunks for the pipelined h1T AllGather (windows per chunk)
WCHUNKS = [(0, 13), (13, 25), (25, 37), (37, 49)]


def _wrap_idx(iv):
    """int16 vector (len%16==0) -> [16, len/16] wrapped layout (compact)."""
    assert len(iv) % 16 == 0
    return iv.reshape(-1, 16).T.copy()    # [16, len/16]


def build_call_slab(iv, chunk):
    """Split iv into chunks (each %16==0, <=chunk), wrap each; concat cols.
    Returns [16, len/16] slab; calls are column ranges."""
    cols = []
    calls = []
    off = 0
    for s in range(0, len(iv), chunk):
        piece = iv[s:s+chunk]
        cols.append(_wrap_idx(piece))
        calls.append((off, len(piece)))
        off += len(piece) // 16
    return np.concatenate(cols, axis=1), calls


def cblob_layout(NBLK):
    """Column offsets of the packed bf16 const blob [128, total]."""
    parts = [('W1', 128), ('B10t', 128), ('Wtab1', 512), ('Wtab2', 512),
             ('Wea1', 256), ('Wea2', 256), ('iota4', 512), ('ident', 128),
             ('dstloc', NBLK), ('deg', WPC), ('grp', WPC)]
    off, lay = 0, {}
    for nm, w in parts:
        lay[nm] = (off, off + w)
        off += w
    return lay, off


FBLOB_LAY = {'invcnt128': (0, 64), 'onehotT': (64, 128), 'fc1a': (128, 160),
             'fc1b': (160, 192), 'fc1bias': (192, 224), 'fc2aug': (224, 225),
             'ones64': (225, 289)}
FBLOB_W = 289


def to_bf16(x):
    import ml_dtypes
    return x.astype(ml_dtypes.bfloat16)


EA_FP8 = True

def to_ea(x):
    import ml_dtypes
    return x.astype(ml_dtypes.float8_e4m3 if EA_FP8 else ml_dtypes.bfloat16)


def prep(inputs):
    x = np.asarray(inputs['x'], np.float32)
    y = np.asarray(inputs['y']).astype(np.int64)
    edge_index = np.asarray(inputs['edge_index']).astype(np.int64)
    ea = np.asarray(inputs['edge_attr'], np.float32)
    batch = np.asarray(inputs['batch']).astype(np.int64)

    src, dst = edge_index[0], edge_index[1]
    cls = y[batch]                      # [N]
    cnt = np.bincount(batch, minlength=G).astype(np.float32)

    # ---- node relabeling: 2D bin-pack nodes by (low,high) in-degree so the
    # per-(core,window) block counts are balanced (shrinks Bw padding).
    # Nodes with old id < S0 stay in positions < S0 (bins 0..254) so each
    # edge's low/high table classification is a fixed constant. ----
    lo_in = np.bincount(dst[src < S0], minlength=NPAD).astype(np.int64)
    hi_in = np.bincount(dst[src >= S0], minlength=NPAD).astype(np.int64)
    perm = np.empty(NPAD, np.int64)
    for ids, bin0, nbins in ((np.arange(S0), 0, S0 // 128),
                             (np.arange(S0, NPAD), S0 // 128,
                              (NPAD - S0) // 128)):
        order = ids[np.argsort(-(lo_in + hi_in)[ids], kind='stable')]
        losum = np.zeros(nbins); hisum = np.zeros(nbins)
        cnts = np.zeros(nbins, np.int64)
        for n in order:
            score = np.maximum((losum + lo_in[n]) / 1328.0,
                               (hisum + hi_in[n]) / 713.0)
            score[cnts >= 128] = np.inf
            b = int(np.argmin(score))
            perm[(bin0 + b) * 128 + cnts[b]] = n
            losum[b] += lo_in[n]; hisum[b] += hi_in[n]; cnts[b] += 1
    invperm = np.empty(NPAD, np.int64)
    invperm[perm] = np.arange(NPAD)
    # keep the relabeling only if it actually reduces Bw
    def _bw_of(s, d):
        ko = (d // NPC * WPC + (d % NPC) // 128) * 2 + (s >= S0)
        cB = np.bincount(ko, minlength=NCORE * WPC * 2)
        lB = -(-cB[0::2].max() // 128); hB = -(-cB[1::2].max() // 128)
        return int(lB), int(hB)
    lb0, hb0 = _bw_of(src, dst)
    lb1, hb1 = _bw_of(invperm[src], invperm[dst])
    if lb1 + hb1 < lb0 + hb0:
        src = invperm[src]; dst = invperm[dst]
    else:
        perm = np.arange(NPAD)

    # ---- per-core edge partition / window / lowhigh split ----
    core_of = dst // NPC
    win_of = (dst % NPC) // 128
    keys = (core_of * WPC + win_of) * 2 + (src >= S0).astype(np.int64)
    orderd = np.argsort(keys, kind='stable')
    ks = keys[orderd]
    bounds = np.searchsorted(ks, np.arange(NCORE * WPC * 2 + 1))
    lowB = 0; highB = 0
    for c in range(NCORE):
        for w in range(WPC):
            k = (c * WPC + w) * 2
            nlo = bounds[k+1] - bounds[k]
            nhi = bounds[k+2] - bounds[k+1]
            lowB = max(lowB, (nlo + 127) // 128)
            highB = max(highB, (nhi + 127) // 128)
    Bw = lowB + highB                  # blocks per window
    SPW = Bw * 128                     # slots per window
    NBLK = WPC * Bw                    # blocks per core
    SL = NBLK * 128                    # slots per core

    # slot arrays per core (idx: no +1 offset; padding idx 0 is gate-killed
    # via dstloc=255 -> zero scatter column)
    srcidx = np.zeros((NCORE, SL), np.int16)
    dstidx = np.zeros((NCORE, SL), np.int16)
    dstloc = np.full((NCORE, SL), 255.0, np.float32)
    easlot = np.zeros((NCORE, SL, D), np.float32)
    for c in range(NCORE):
        for w in range(WPC):
            k = (c * WPC + w) * 2
            elo = orderd[bounds[k]:bounds[k+1]]
            ehi = orderd[bounds[k+1]:bounds[k+2]]
            base = (w * Bw) * 128
            srcidx[c, base:base+len(elo)] = src[elo].astype(np.int16)
            dstidx[c, base:base+len(elo)] = (dst[elo] - c * NPC).astype(np.int16)
            dstloc[c, base:base+len(elo)] = (dst[elo] % 128).astype(np.float32)
            easlot[c, base:base+len(elo)] = ea[elo]
            hbase = base + lowB * 128
            srcidx[c, hbase:hbase+len(ehi)] = (src[ehi] - S0).astype(np.int16)
            dstidx[c, hbase:hbase+len(ehi)] = (dst[ehi] - c * NPC).astype(np.int16)
            dstloc[c, hbase:hbase+len(ehi)] = (dst[ehi] % 128).astype(np.float32)
            easlot[c, hbase:hbase+len(ehi)] = ea[ehi]

    # gather slabs (per core, compact [16, n/16]): per window, calls over
    # low slots / high slots / dst slots
    CH = 1024
    src_slabs, dst_slabs = [], []
    for c in range(NCORE):
        scols, dcols = [], []
        src_calls, dst_calls = [], []   # per window: list of (coloff, n, tblsel)
        for w in range(WPC):
            base = w * SPW
            lo = srcidx[c, base:base+lowB*128]
            hi = srcidx[c, base+lowB*128:base+SPW]
            sl, cl = build_call_slab(lo, CH)
            off0 = sum(s.shape[1] for s in scols)
            scols.append(sl)
            wcalls = [(off0+o, n, 0) for (o, n) in cl]
            sl, cl = build_call_slab(hi, CH)
            off0 = sum(s.shape[1] for s in scols)
            scols.append(sl)
            wcalls += [(off0+o, n, 1) for (o, n) in cl]
            src_calls.append(wcalls)
            dl, cl = build_call_slab(dstidx[c, base:base+SPW], CH)
            off0 = sum(d.shape[1] for d in dcols)
            dcols.append(dl)
            dst_calls.append([(off0+o, n) for (o, n) in cl])
        src_slabs.append(np.concatenate(scols, axis=1))
        dst_slabs.append(np.concatenate(dcols, axis=1))
    src_slab = np.stack(src_slabs)      # [NCORE, 16, SL/16]
    dst_slab = np.stack(dst_slabs)

    # dstloc arranged [128, NBLK]: slot i -> [i%128, i//128]
    dstloc_a = dstloc.reshape(NCORE, NBLK, 128).transpose(0, 2, 1)  # [NCORE,128,NBLK]
    # eaT [WPC*33, Bw*128]: row w*33+p, col b*128+j = easlot[c, (w*Bw+b)*128+j, p]
    eaT = np.ones((NCORE, WPC, 33, Bw * 128), np.float32)
    eaT[:, :, :32, :] = easlot.reshape(NCORE, WPC, Bw, 128, D).transpose(
        0, 1, 4, 2, 3).reshape(NCORE, WPC, D, Bw * 128)

    # phase A (own-only); node-position arrays follow the relabeling perm
    xT_full = np.zeros((C, NPAD), np.float32); xT_full[:, :N] = x.T
    xT_full = xT_full[:, perm]
    clsidx = np.zeros(NPAD, np.int16); clsidx[:N] = cls + 1
    clsOH = np.zeros((11, NPAD), np.float32)
    clsOH[clsidx, np.arange(NPAD)] = 1.0
    clsOH = clsOH[:, perm]

    degp = np.bincount(dst, minlength=NPAD).astype(np.float32)
    deg_a = degp.reshape(NCORE, WPC, 128).transpose(0, 2, 1)  # [NCORE,128,WPC]

    # group id per own node, staircase layout [128, WPC]; pad -> 255
    gl = np.full(NPAD, 255, np.int64); gl[:N] = batch
    gl = gl[perm]
    grp_a = gl.reshape(NCORE, WPC, 128).transpose(0, 2, 1).astype(np.float32)
    invcnt = (1.0 / np.maximum(cnt, 1.0)).astype(np.float32)
    invcnt128 = np.tile(invcnt[None, :], (128, 1)).copy()   # [128, G]

    W = {k: np.asarray(inputs[k], np.float32) for k in
         ['lin_W','lin_b','c1_Wf','c1_bf','c1_Ws','c1_bs','c2_Wf','c2_bf','c2_Ws','c2_bs',
          'fc1_W','fc1_b','fc2_W','fc2_b']}
    B10 = np.zeros((11, C), np.float32); B10[1:] = W['lin_W'][C:] + W['lin_b']
    AL = 0.84154   # 0.42077/0.5: lets one Tanh(0.5*x) op cover f and s halves
    def wtab(k):
        Wf, Ws = W[f'c{k}_Wf'], W[f'c{k}_Ws']
        return np.concatenate([Ws[:C] * AL, Wf[:C],
                               Ws[C:2*C] * AL, Wf[C:2*C]], axis=1)  # [128,512]
    def wea(k):
        Wf, Ws, bf, bs = W[f'c{k}_Wf'], W[f'c{k}_Ws'], W[f'c{k}_bf'], W[f'c{k}_bs']
        m = np.concatenate([Ws[2*C:] * AL, Wf[2*C:]], axis=1)      # [32,256]
        return np.concatenate([m, np.concatenate([bs * AL, bf])[None, :]], axis=0)  # [33,256]

    iota4 = np.tile(np.arange(128, dtype=np.float32), 4)[None, :].repeat(128, 0)  # [128,512]
    onehotT = np.zeros((NCLS, G), np.float32)
    for g in range(G): onehotT[y[g], g] = 1.0

    meta = dict(lowB=lowB, highB=highB, Bw=Bw, SPW=SPW, NBLK=NBLK, SL=SL,
                src_calls=src_calls, dst_calls=dst_calls)

    # pack all small constants into consolidated blobs: per-tensor transfer
    # overhead on this runtime is ~0.16 ms, so fewer inputs wins.
    lay, CBW = cblob_layout(NBLK)
    def put(blob, lay_d, nm, arr):
        o0, o1 = lay_d[nm]
        blob[:arr.shape[0], o0:o1] = arr
    fblob = np.zeros((128, FBLOB_W), np.float32)
    put(fblob, FBLOB_LAY, 'invcnt128', invcnt128)
    put(fblob, FBLOB_LAY, 'onehotT', onehotT)
    put(fblob, FBLOB_LAY, 'fc1a', W['fc1_W'][:C])
    put(fblob, FBLOB_LAY, 'fc1b', W['fc1_W'][C:])
    put(fblob, FBLOB_LAY, 'fc1bias', W['fc1_b'][None, :])
    put(fblob, FBLOB_LAY, 'fc2aug',
        np.concatenate([W['fc2_W'], W['fc2_b'][None, :]], 0))
    put(fblob, FBLOB_LAY, 'ones64', np.ones((1, G), np.float32))

    per_core = []
    for c in range(NCORE):
        cb = np.zeros((128, CBW), np.float32)
        put(cb, lay, 'W1', W['lin_W'][:C])
        put(cb, lay, 'B10t', B10)
        put(cb, lay, 'Wtab1', wtab(1)); put(cb, lay, 'Wtab2', wtab(2))
        put(cb, lay, 'Wea1', wea(1)); put(cb, lay, 'Wea2', wea(2))
        put(cb, lay, 'iota4', iota4)
        put(cb, lay, 'ident', np.eye(128, dtype=np.float32))
        put(cb, lay, 'dstloc', dstloc_a[c])
        put(cb, lay, 'deg', deg_a[c]); put(cb, lay, 'grp', grp_a[c])
        m = {
            'xT_own': to_ea(xT_full[:, c*NPC:(c+1)*NPC]),
            'clsOH_own': to_bf16(clsOH[:, c*NPC:(c+1)*NPC]),
            'eaT': to_ea(eaT[c].reshape(WPC * 33, Bw * 128)),
            'islab': np.concatenate([src_slab[c], dst_slab[c]], axis=1),
            'cblob': to_bf16(cb),
            'fblob': fblob,
        }
        per_core.append(m)
    return per_core, meta



# ======================= bass kernel builder =======================

import concourse.bass as bass
import concourse.bacc as bacc
import concourse.tile as tile
from concourse import mybir, library_config

F32 = mybir.dt.float32
BF16 = mybir.dt.bfloat16
FP8 = mybir.dt.float8e4
EADT = FP8 if EA_FP8 else BF16
I16 = mybir.dt.int16
AF = mybir.ActivationFunctionType

N, E, C, D, NCLS, G = 50000, 800000, 128, 32, 10, 64
NCORE, NPC, WPC = 8, 6272, 49
NPAD = NCORE * NPC
S0 = 32640
NTAB = NPAD // 128          # 392 table blocks


def build(meta, stage="full"):
    lowB, highB, Bw = meta['lowB'], meta['highB'], meta['Bw']
    SPW = Bw * 128
    NBLK = WPC * Bw
    SL = NBLK * 128
    src_calls = meta['src_calls']
    dst_calls = meta['dst_calls']

    nc = bacc.Bacc("TRN2", target_bir_lowering=False, debug=False,
                   num_devices=NCORE, num_swdge_queues=4)

    def inp(name, shape, dt):
        return nc.dram_tensor(name, shape, dt, kind="ExternalInput")

    lay, CBW = cblob_layout(NBLK)
    xT_own = inp("xT_own", [C, NPC], EADT)
    clsOHo_in = inp("clsOH_own", [11, NPC], BF16)
    eaT_in = inp("eaT", [WPC * 33, Bw * 128], EADT)
    islab_in = inp("islab", [16, 2 * (SL // 16)], I16)
    cblob_in = inp("cblob", [128, CBW], BF16)
    fblob_in = inp("fblob", [128, FBLOB_W], F32)

    out_t = nc.dram_tensor("out", [G, 1], F32, kind="ExternalOutput")
    dbg = {}
    if stage == "h0":
        dbg['h0_own'] = nc.dram_tensor("dbg_h0", [NPC, C], BF16, kind="ExternalOutput")
    if stage in ("h1", "full"):
        dbg['h1_own'] = nc.dram_tensor("dbg_h1", [NPC, C], BF16, kind="ExternalOutput")
    if stage == "full":
        dbg['h2_own'] = nc.dram_tensor("dbg_h2", [NPC, C], BF16, kind="ExternalOutput")

    with tile.TileContext(nc) as tc:
        nc.gpsimd.load_library(library_config.mlp)
        ctx = contextlib.ExitStack()
        consts = ctx.enter_context(tc.tile_pool(name="consts", bufs=1))
        sbuf = ctx.enter_context(tc.tile_pool(name="sbuf", bufs=2))
        sbuf3 = ctx.enter_context(tc.tile_pool(name="sbuf3", bufs=3))
        gates = ctx.enter_context(tc.tile_pool(name="gates", bufs=4))
        dram = ctx.enter_context(tc.tile_pool(name="dram", bufs=1, space="DRAM"))

        CB = consts.tile([128, CBW], BF16, tag="cblob_c")
        nc.sync.dma_start(CB[:], cblob_in[:])
        def csl(nm, rows=128):
            o0, o1 = lay[nm]
            return CB[0:rows, o0:o1]
        W1 = csl('W1')
        iota4 = csl('iota4')
        ident = csl('ident')
        Wtab = [csl('Wtab1'), csl('Wtab2')]
        Wea = [csl('Wea1', 33), csl('Wea2', 33)]
        dstlocC = csl('dstloc')
        degC = csl('deg')
        grpC = csl('grp')
        B10sb = csl('B10t', 11)
        clsOHsb = consts.tile([11, NPC], BF16, tag="clsOH_c")
        nc.sync.dma_start(clsOHsb[:], clsOHo_in[:])
        FB = consts.tile([128, FBLOB_W], F32, tag="fblob_c")
        nc.sync.dma_start(FB[:], fblob_in[:])
        def fsl(nm, rows=128):
            o0, o1 = FBLOB_LAY[nm]
            return FB[0:rows, o0:o1]
        # expand de-replicated index slabs [16, n] -> [128, n]
        srcsl = consts.tile([128, SL // 16], I16, tag="srcsl")
        dstsl = consts.tile([128, SL // 16], I16, tag="dstsl")
        for kk in range(8):
            nc.sync.dma_start(srcsl[16 * kk:16 * kk + 16, :],
                              islab_in[:, 0:SL // 16])
            nc.sync.dma_start(dstsl[16 * kk:16 * kk + 16, :],
                              islab_in[:, SL // 16:])

        h0ownT = consts.tile([C, NPC], BF16, tag="h0ownT")
        h0own = consts.tile([128, WPC * 128], BF16, tag="h0own")
        h1own = consts.tile([128, WPC * 128], BF16, tag="h1own")

        # DRAM tables (no +1 pad row: padded slots gather garbage that the
        # zero scatter column kills)
        ts1_own = dram.tile([NPC, 256], BF16)
        TS1 = dram.tile([NPAD, 256], BF16)
        ts2A = dram.tile([S0, 256], BF16)
        ts2B = dram.tile([NPAD - S0, 256], BF16)
        tdL = [dram.tile([NPC, 256], BF16, name=f"tdL{_k}", tag=f"tdL{_k}")
               for _k in range(2)]
        h1T_c = [dram.tile([C, (w1 - w0) * 128], BF16, name=f"h1Tc{ci}",
                           tag=f"h1Tc{ci}")
                 for ci, (w0, w1) in enumerate(WCHUNKS)]
        blob2 = [dram.tile([NCORE * C, (w1 - w0) * 128], BF16,
                           name=f"blob2{ci}", tag=f"blob2{ci}")
                 for ci, (w0, w1) in enumerate(WCHUNKS)]
        ar_in = dram.tile([128, G], F32)
        ar_out = dram.tile([128, G], F32)

        qn = [0]
        def next_q():
            q = qn[0] % 4
            qn[0] += 1
            return q

        # ================= PHASE A (own nodes only) =================
        # pass 1: h0ownT + ts1 rows (feeds the AllGather ASAP); pass 2
        # (transpose/h0own/td1) runs under the collective.
        with tc.tile_pool(name="psA", bufs=2, space="PSUM") as psA, \
             tc.tile_pool(name="psT1", bufs=2, space="PSUM") as psT1:
            for w in range(WPC):
                xt = sbuf.tile([128, 128], EADT, tag="pAx")
                nc.sync.dma_start(xt[:], xT_own[:, w * 128:(w + 1) * 128])
                ps = psA.tile([128, 128], F32, tag="pA")
                nc.tensor.matmul(out=ps[:], lhsT=W1, rhs=xt[:],
                                 start=True, stop=False)
                nc.tensor.matmul(out=ps[:], lhsT=B10sb[:],
                                 rhs=clsOHsb[:, w * 128:(w + 1) * 128],
                                 start=False, stop=True)
                nc.scalar.activation(h0ownT[:, w * 128:(w + 1) * 128], ps[:],
                                     AF.Prelu, alpha=0.01)
                ps3 = psT1.tile([128, 256], F32, tag="ts1")
                nc.tensor.matmul(out=ps3[:],
                                 lhsT=h0ownT[:, w * 128:(w + 1) * 128],
                                 rhs=Wtab[0][:, 256:512], start=True, stop=True)
                ev2 = sbuf.tile([128, 256], BF16, tag="ts1ev")
                nc.vector.tensor_copy(ev2[:], ps3[:])
                nc.sync.dma_start(ts1_own[w * 128:(w + 1) * 128, :], ev2[:])

            # AllGather per-core src-table rows -> full conv1 table
            nc.gpsimd.collective_compute(
                "AllGather", mybir.AluOpType.bypass,
                replica_groups=[list(range(NCORE))],
                ins=[ts1_own.opt()], outs=[TS1.opt()])

            for w in range(WPC):
                pst = psA.tile([128, 128], BF16, tag="pAtr")
                nc.tensor.transpose(pst[:], h0ownT[:, w * 128:(w + 1) * 128],
                                    ident)
                nc.vector.tensor_copy(h0own[:, w * 128:(w + 1) * 128], pst[:])
                ps2 = psT1.tile([128, 256], F32, tag="td1")
                nc.tensor.matmul(out=ps2[:],
                                 lhsT=h0ownT[:, w * 128:(w + 1) * 128],
                                 rhs=Wtab[0][:, 0:256], start=True, stop=True)
                ev = sbuf.tile([128, 256], BF16, tag="td1ev")
                nc.scalar.activation(ev[:], ps2[:], AF.Copy)
                nc.sync.dma_start(tdL[0][w * 128:(w + 1) * 128, :], ev[:])

        if stage == "h0":
            for w in range(WPC):
                nc.sync.dma_start(dbg['h0_own'][w * 128:(w + 1) * 128, :],
                                  h0own[:, w * 128:(w + 1) * 128])
            ctx.close()
            return nc, dbg

        # ================= CONV =================
        def conv(k, hprev_own, hout_own, leaky, psGate, psAgg, psFlush,
                 pool_mm, tsA_ap, tsB_ap, post_window=None):
            for w in range(WPC):
                tsg = sbuf.tile([128, Bw, 256], BF16, tag="tsg")
                tdg = sbuf.tile([128, Bw, 256], BF16, tag="tdg")
                base16 = w * (SPW // 16)
                for (aoff, n, tbl) in src_calls[w]:
                    s0 = (aoff - base16) * 16
                    nc.gpsimd.dma_gather(
                        out_ap=tsg[:, s0 // 128: s0 // 128 + n // 128, :],
                        in_ap=(tsA_ap if tbl == 0 else tsB_ap),
                        idxs_ap=srcsl[:, aoff:aoff + n // 16],
                        num_idxs=n, num_idxs_reg=n, elem_size=256,
                        queue_num=next_q())
                for (aoff, n) in dst_calls[w]:
                    s0 = (aoff - base16) * 16
                    nc.gpsimd.dma_gather(
                        out_ap=tdg[:, s0 // 128: s0 // 128 + n // 128, :],
                        in_ap=tdL[k][:],
                        idxs_ap=dstsl[:, aoff:aoff + n // 16],
                        num_idxs=n, num_idxs_reg=n, elem_size=256,
                        queue_num=next_q())
                eaw = sbuf.tile([33, Bw * 128], EADT, tag="eaw")
                nc.sync.dma_start(eaw[:], eaT_in[w * 33:(w + 1) * 33, :])
                dlw = dstlocC[:, w * Bw:(w + 1) * Bw]

                agg = psAgg.tile([128, 256], F32, tag="agg")
                for g0 in range(0, Bw, 4):
                    ng = min(4, Bw - g0)
                    ps = psGate.tile([128, 1024], F32, tag="gate")
                    for b in range(ng):
                        blk = g0 + b
                        # z = ea@Wea + Ts[src] + Td[dst] accumulated in PSUM
                        # (identity matmuls add the gathered table tiles)
                        nc.tensor.matmul(
                            out=ps[:, b * 256:(b + 1) * 256],
                            lhsT=eaw[:, blk * 128:(blk + 1) * 128],
                            rhs=Wea[k], start=True, stop=False)
                        nc.tensor.matmul(
                            out=ps[:, b * 256:(b + 1) * 256],
                            lhsT=ident, rhs=tsg[:, blk, :],
                            start=False, stop=False)
                        nc.tensor.matmul(
                            out=ps[:, b * 256:(b + 1) * 256],
                            lhsT=ident, rhs=tdg[:, blk, :],
                            start=False, stop=True)
                    psv = ps[:].rearrange("p (b e) -> p b e", e=256)
                    sgt = gates.tile([128, 4, 128], BF16, tag="sgt")
                    nc.vector.tensor_tensor(
                        out=sgt[:, :ng, :],
                        in0=dlw[:, g0:g0 + ng].rearrange(
                            "p (b o) -> p b o", o=1).to_broadcast([128, ng, 128]),
                        in1=iota4[:, :512].rearrange("p (b e) -> p b e", b=4)[:, :ng, :],
                        op=mybir.AluOpType.is_equal)
                    # psum cols per block: [s'(0:128) || f(128:256)]; one tanh
                    # covers both halves (s pre-scaled by 0.84154 on host)
                    wta = gates.tile([128, 4, 256], BF16, tag="wta")
                    nc.scalar.activation(
                        wta[:, :ng, :].rearrange("p b e -> p (b e)"),
                        ps[:, :ng * 256], AF.Tanh, scale=0.5)
                    sl = gates.tile([128, 4, 128], BF16, tag="sl")
                    nc.scalar.activation(sl[:, :ng, :], psv[:, :ng, 0:128],
                                         AF.Silu, scale=1.18829)
                    sq = gates.tile([128, 4, 128], BF16, tag="sq")
                    nc.vector.tensor_tensor(
                        out=sq[:, :ng, :], in0=wta[:, :ng, 0:128],
                        in1=wta[:, :ng, 0:128], op=mybir.AluOpType.mult)
                    v = gates.tile([128, 4, 128], BF16, tag="v")
                    nc.vector.scalar_tensor_tensor(
                        out=v[:, :ng, :], in0=sq[:, :ng, :], scalar=-0.69217,
                        in1=sl[:, :ng, :], op0=mybir.AluOpType.mult,
                        op1=mybir.AluOpType.add)
                    nc.vector.scalar_tensor_tensor(
                        out=wta[:, :ng, 0:128],
                        in0=wta[:, :ng, 128:256], scalar=1.0,
                        in1=v[:, :ng, :], op0=mybir.AluOpType.add,
                        op1=mybir.AluOpType.mult)
                    for b in range(ng):
                        blk = g0 + b
                        nc.tensor.matmul(
                            out=agg[:], lhsT=sgt[:, b, :],
                            rhs=wta[:, b, :],
                            start=(blk == 0), stop=(blk == Bw - 1))
                t2 = sbuf.tile([128, 128], F32, tag="t2")
                nc.vector.scalar_tensor_tensor(
                    out=t2[:], in0=degC[:, w:w + 1].to_broadcast([128, 128]),
                    scalar=0.34609, in1=hprev_own[:, w * 128:(w + 1) * 128],
                    op0=mybir.AluOpType.mult, op1=mybir.AluOpType.add)
                t3 = sbuf.tile([128, 128], F32, tag="t3")
                nc.vector.scalar_tensor_tensor(
                    out=t3[:], in0=agg[:, 128:256], scalar=0.34609,
                    in1=t2[:], op0=mybir.AluOpType.mult,
                    op1=mybir.AluOpType.add)
                hsum = sbuf.tile([128, 128], F32, tag="hsum")
                nc.vector.scalar_tensor_tensor(
                    out=hsum[:], in0=agg[:, 0:128], scalar=0.5,
                    in1=t3[:], op0=mybir.AluOpType.mult,
                    op1=mybir.AluOpType.add)
                hw = sbuf.tile([128, 128], BF16, tag="hw")
                if leaky:
                    nc.scalar.activation(hw[:], hsum[:], AF.Prelu, alpha=0.01)
                else:
                    nc.scalar.activation(hw[:], hsum[:], AF.Copy)
                nc.vector.tensor_copy(hout_own[:, w * 128:(w + 1) * 128], hw[:])
                if k == 0:
                    # transpose -> h1T chunk (for AllGather) + own td2 rows
                    pst = psFlush.tile([128, 128], BF16, tag="flushtr")
                    nc.tensor.transpose(pst[:], hw[:], ident)
                    h1T = sbuf.tile([128, 128], BF16, tag="h1T")
                    nc.scalar.activation(h1T[:], pst[:], AF.Copy)
                    ci = next(i for i, (w0, w1) in enumerate(WCHUNKS)
                              if w0 <= w < w1)
                    w0 = WCHUNKS[ci][0]
                    nc.sync.dma_start(
                        h1T_c[ci][:, (w - w0) * 128:(w - w0 + 1) * 128], h1T[:])
                    ps2 = psFlush.tile([128, 256], F32, tag="td2")
                    nc.tensor.matmul(out=ps2[:], lhsT=h1T[:],
                                     rhs=Wtab[1][:, 0:256], start=True, stop=True)
                    ev = sbuf.tile([128, 256], BF16, tag="td2ev")
                    nc.scalar.activation(ev[:], ps2[:], AF.Copy)
                    nc.sync.dma_start(tdL[1][w * 128:(w + 1) * 128, :], ev[:])
                else:
                    sbw = sbuf.tile([128, G], BF16, tag="sbw")
                    nc.vector.tensor_tensor(
                        out=sbw[:],
                        in0=grpC[:, w:w + 1].to_broadcast([128, G]),
                        in1=iota4[:, 0:G],
                        op=mybir.AluOpType.is_equal)
                    nc.tensor.matmul(out=pool_mm[:, :G], lhsT=hw[:],
                                     rhs=sbw[:],
                                     start=(w == 0), stop=(w == WPC - 1))
                if post_window is not None:
                    post_window(w)

        def fire_ag2(w):
            for ci, (w0, w1) in enumerate(WCHUNKS):
                if w == w1 - 1:
                    nc.gpsimd.collective_compute(
                        "AllGather", mybir.AluOpType.bypass,
                        replica_groups=[list(range(NCORE))],
                        ins=[h1T_c[ci].opt()], outs=[blob2[ci].opt()])

        # conv2 src-table rows from the gathered h1T blobs; interleaved into
        # conv1's window loop so the blob-read DMAs never sit blocked in the
        # in-order SP sequencer stream ahead of conv1's own DMAs.
        psT2 = ctx.enter_context(tc.tile_pool(name="psT2", bufs=1, space="PSUM"))

        def table2_chunk(ci):
            w0, w1 = WCHUNKS[ci]
            ncol = (w1 - w0) * 128
            for cb in range(NCORE):
                bt = sbuf3.tile([128, 13 * 128], BF16, tag="btile")
                nc.sync.dma_start(bt[:, :ncol],
                                  blob2[ci][cb * C:(cb + 1) * C, :])
                nbs, nbe = cb * WPC + w0, cb * WPC + w1
                for nb0 in range(nbs, nbe, 2):
                    nblk = min(2, nbe - nb0)
                    ev = sbuf.tile([128, 512], BF16, tag="tmmev")
                    for jj in range(nblk):
                        nb = nb0 + jj
                        w = nb % WPC
                        ps = psT2.tile([128, 256], F32, tag="tmm")
                        nc.tensor.matmul(
                            out=ps[:], lhsT=bt[:, (w - w0) * 128:(w - w0 + 1) * 128],
                            rhs=Wtab[1][:, 256:512], start=True, stop=True)
                        if (nb0 + jj) % 2 == 0:
                            nc.scalar.activation(
                                ev[:, jj * 256:(jj + 1) * 256], ps[:], AF.Copy)
                        else:
                            nc.vector.tensor_copy(
                                ev[:, jj * 256:(jj + 1) * 256], ps[:])
                    if nblk == 2:
                        evv = ev[:].rearrange("p (b e) -> p b e", b=2)
                        if nb0 < 254:
                            nc.sync.dma_start(
                                ts2A[nb0 * 128:(nb0 + 2) * 128, :].rearrange(
                                    "(b p) e -> p b e", b=2), evv)
                        elif nb0 >= 255:
                            b = nb0 - 255
                            nc.sync.dma_start(
                                ts2B[b * 128:(b + 2) * 128, :].rearrange(
                                    "(b p) e -> p b e", b=2), evv)
                        else:
                            nc.sync.dma_start(
                                ts2A[254 * 128:255 * 128, :], ev[:, 0:256])
                            nc.sync.dma_start(ts2B[0:128, :], ev[:, 256:512])
                    else:
                        if nb0 < 255:
                            nc.sync.dma_start(
                                ts2A[nb0 * 128:(nb0 + 1) * 128, :], ev[:, 0:256])
                        else:
                            nc.sync.dma_start(
                                ts2B[(nb0 - 255) * 128:(nb0 - 254) * 128, :],
                                ev[:, 0:256])

        def post_w0(w):
            fire_ag2(w)
            if w == 25:
                table2_chunk(0)
            elif w == 37:
                table2_chunk(1)
            elif w == 48:
                table2_chunk(2)

        with tc.tile_pool(name="psG1", bufs=2, space="PSUM") as psG1, \
             tc.tile_pool(name="psA1", bufs=1, space="PSUM") as psA1, \
             tc.tile_pool(name="psF1", bufs=1, space="PSUM") as psF1:
            conv(0, h0own, h1own, True, psG1, psA1, psF1, None,
                 TS1[0:S0], TS1[S0:NPAD], post_window=post_w0)

        if stage == "h1":
            for w in range(WPC):
                nc.sync.dma_start(dbg['h1_own'][w * 128:(w + 1) * 128, :],
                                  h1own[:, w * 128:(w + 1) * 128])
            ctx.close()
            return nc, dbg

        table2_chunk(3)

        h2own = h0own
        with tc.tile_pool(name="psPool", bufs=1, space="PSUM") as psPool:
            pool_mm = psPool.tile([128, G], F32, tag="pool")
            with tc.tile_pool(name="psG2", bufs=2, space="PSUM") as psG2, \
                 tc.tile_pool(name="psA2", bufs=1, space="PSUM") as psA2:
                conv(1, h1own, h2own, False, psG2, psA2, None, pool_mm,
                     ts2A[:], ts2B[:])

            if stage == "full":
                for w in range(WPC):
                    nc.sync.dma_start(dbg['h1_own'][w * 128:(w + 1) * 128, :],
                                      h1own[:, w * 128:(w + 1) * 128])
                    nc.sync.dma_start(dbg['h2_own'][w * 128:(w + 1) * 128, :],
                                      h2own[:, w * 128:(w + 1) * 128])

            poolsb = sbuf.tile([128, G], F32, tag="poolsb")
            nc.vector.tensor_copy(poolsb[:], pool_mm[:])
        nc.sync.dma_start(ar_in[:], poolsb[:])
        nc.gpsimd.collective_compute(
            "AllReduce", mybir.AluOpType.add,
            replica_groups=[list(range(NCORE))],
            ins=[ar_in.opt()], outs=[ar_out.opt()])

        with tc.tile_pool(name="psH", bufs=1, space="PSUM") as psH:
            pooled = sbuf.tile([128, G], F32, tag="pooled")
            nc.sync.dma_start(pooled[:], ar_out[:])
            nc.vector.tensor_tensor(out=pooled[:], in0=pooled[:],
                                    in1=fsl('invcnt128'),
                                    op=mybir.AluOpType.mult)
            fc1a = fsl('fc1a')
            fc1b = fsl('fc1b', NCLS)
            fc1bias = fsl('fc1bias', 1)
            fc2aug = fsl('fc2aug', 33)
            ones64 = fsl('ones64', 1)
            onehotT = fsl('onehotT', NCLS)
            hps = psH.tile([32, G], F32, tag="head1")
            nc.tensor.matmul(out=hps[:], lhsT=fc1a, rhs=pooled[:],
                             start=True, stop=False)
            nc.tensor.matmul(out=hps[:], lhsT=fc1b, rhs=onehotT,
                             start=False, stop=False)
            nc.tensor.matmul(out=hps[:], lhsT=fc1bias, rhs=ones64,
                             start=False, stop=True)
            a1 = sbuf.tile([33, G], F32, tag="a1")
            nc.scalar.activation(a1[0:32, :], hps[:], AF.Prelu, alpha=0.01)
            nc.vector.memset(a1[32:33, :], 1.0)
            hps2 = psH.tile([1, G], F32, tag="head2")
            nc.tensor.matmul(out=hps2[:], lhsT=fc2aug, rhs=a1[:],
                             start=True, stop=True)
            rest = sbuf.tile([1, G], F32, tag="rest")
            nc.scalar.activation(rest[:], hps2[:], AF.Tanh, scale=0.5)
            res = sbuf.tile([1, G], F32, tag="res")
            nc.vector.tensor_scalar(res[:], rest[:], 0.5, 0.5,
                                    mybir.AluOpType.mult,
                                    mybir.AluOpType.add)
            nc.sync.dma_start(out_t[:].rearrange("g o -> o g"), res[:])

        ctx.close()
    return nc, dbg


# ======================= entry point =======================
_CACHE = {}

def _get_compiled(meta_key, meta):
    if meta_key not in _CACHE:
        nc, _ = build(meta, stage="final")
        nc.compile()
        _CACHE[meta_key] = nc
    return _CACHE[meta_key]


def make_inputs(inputs):
    return prep(inputs)


def kernel(**inputs) -> np.ndarray:
    per_core, meta = make_inputs(inputs)
    key = (meta['lowB'], meta['highB'])
    nc = _get_compiled(key, meta)
    from concourse.bass_utils import run_bass_kernel_spmd
    res = run_bass_kernel_spmd(nc, per_core, core_ids=list(range(NCORE)))
    return np.asarray(res.results[0]['out'], dtype=np.float32)
